# revision 10
# baseline (speedup 1.0000x reference)
"""Trainium2 Bass kernel for a custom-activation LSTM cell.

  gates = (x @ w_ih.T + b_ih) + (h @ w_hh.T + b_hh)   # [B, 4H], gate order f,i,ic,o
  ft, it, ot = sigmoid(...), i_cands = sin(ic_in)
  ct = c*ft + sin(ic_in)*it ; ht = sigmoid(o_in)*sin(ct)

Sharding: each of the 8 cores computes the SAME 256-wide slice of H for all
four gates (rows g*2048 + core*256 .. +256 of the weight matrices). Each core
then owns columns [core*256, (core+1)*256) of ht/ct for the full batch — no
cross-core communication is needed.

Matmul precision/speed: fp8(e4m3) DoubleRow runs the PE at 2x the fp32r rate
(0.5 cycles/row). Raw fp8 quantization error (~3e-2) exceeds the 2e-2
tolerance, so the kernel uses residual compensation:

  x ~= (X + S)/32,  W ~= (W' + R)/256   with X=e4m3(32x), S=e4m3(32x - X),
                                             W'=e4m3(256w), R=e4m3(256w - W')
  x@w ~= (X@W' + S@W' + X@R) / 8192     (drop S@R, ~1e-3 rel)

All three terms share the PSUM scale kappa=8192 because main and residual
planes share one scale family, so they accumulate into ONE PSUM bank.
DoubleRow packs two K=128 rows per instruction:
  - main X@W': k-tile pairs (2kp, 2kp+1)            -> 12 instructions
  - correction: per k-tile the pair (W'.S ; R.X)    -> 24 instructions
36 half-rate instructions vs fp32r's 24 full-rate = 0.75x PE time, with
measured end-to-end error ~9e-4 (20x inside the 2e-2 gate).

The 1/kappa descale folds into the ACT scale: sigmoid(z+b) =
0.5*tanh((0.5/kappa)*psum + 0.5b) + 0.5; for the sin gate the range wrap runs
on kappa-scaled values (shift=kappa*b, bound=kappa*pi, period=kappa*2pi) and
ACT Sin applies scale=1/kappa. Tanh and Sin come from ONE ACT table set
(silu_and_others, forced by a leading Silu) to avoid ~2.7us table switches.
"""

import numpy as np
import ml_dtypes

import concourse.bass as bass
import concourse.tile as tile
from concourse import bacc, mybir
from concourse.bass_utils import run_bass_kernel_spmd

# Problem shapes (hardcoded per the harness contract).
B, IN, H = 4096, 1024, 2048
NCORES = 8
P = 128
SH = H // NCORES          # 256  H-slice per core
G = 4 * SH                # 1024 gate rows per core (f,i,ic,o x 256)
MT = G // P               # 8 m-tiles: [f0 i0 ic0 o0 | f1 i1 ic1 o1]
KX = IN // P              # 8 k-tiles from x
KH = H // P               # 16 k-tiles from h
NB = 512                  # batch chunk (PSUM bank = 512 fp32)
NBCH = B // NB            # 8 chunks

KP = (KX + KH) // 2       # 12 main k-tile pairs
KT = KX + KH              # 24 correction k-tiles

AQ = 32.0                 # activation pre-scale into fp8
WQ = 256.0                # weight pre-scale into fp8
KAPPA = AQ * WQ           # PSUM scale of every matmul term

F32 = mybir.dt.float32
E4 = mybir.dt.float8e4
E4NP = ml_dtypes.float8_e4m3   # numpy dtype matching mybir.dt.np(float8e4)
ACT = mybir.ActivationFunctionType
SWIL = mybir.MatmulPerfMode.DoubleRowSwInterleave

_MODULES: dict[int, "bacc.Bacc"] = {}


def _build_module(repeats: int = 1, lead_silu: bool = True,
                  internal_io: bool = False) -> "bacc.Bacc":
    """Build + compile the per-core Bass module.

    repeats > 1 wraps the whole compute in a hardware loop (used only for
    timing: the per-iteration device time is (T(R) - T(1)) / (R - 1))."""
    nc = bacc.Bacc("TRN2", target_bir_lowering=False, debug=False,
                   num_devices=NCORES)

    # internal_io=True is a timing-only variant: the big tensors live in
    # Internal DRAM (uninitialized, never uploaded/downloaded) so the
    # per-call wall time is not dominated by host<->device transfers.
    kin = "Internal" if internal_io else "ExternalInput"
    kout = "Internal" if internal_io else "ExternalOutput"

    # Row order of xq/hq/w*q: (k-tile, plane, partition). Act planes:
    # 0 = S (residual), 1 = X (main). Weight planes: 0 = W' (main), 1 = R.
    xq = nc.dram_tensor("xq", [KX * 2 * P, B], E4, kind=kin).ap()
    hq = nc.dram_tensor("hq", [KH * 2 * P, B], E4, kind=kin).ap()
    cT = nc.dram_tensor("cT", [SH, B], F32, kind=kin).ap()
    # SwInterleave stationaries: wm = main pairs (W'[2kp], W'[2kp+1]),
    # wc = correction pairs (W'[kt], R[kt]); rows (ktile, p), cols (m, 256).
    wm = nc.dram_tensor("wm", [KP * P, MT * 256], E4, kind=kin).ap()
    wc = nc.dram_tensor("wc", [KT * P, MT * 256], E4, kind=kin).ap()
    biasd = nc.dram_tensor("biasd", [P, MT], F32, kind="ExternalInput").ap()
    htT = nc.dram_tensor("htT", [SH, B], F32, kind=kout).ap()
    ctT = nc.dram_tensor("ctT", [SH, B], F32, kind=kout).ap()

    xq4 = xq.rearrange("(ko two p) b -> p ko two b", p=P, two=2)
    hq4 = hq.rearrange("(ko two p) b -> p ko two b", p=P, two=2)
    cT3 = cT.rearrange("(po p) b -> p po b", p=P)       # [128, 2, B]
    wm3 = wm.rearrange("(kp p) f -> p kp f", p=P)
    wc3 = wc.rearrange("(kt p) f -> p kt f", p=P)
    htT3 = htT.rearrange("(po p) b -> p po b", p=P)
    ctT3 = ctT.rearrange("(po p) b -> p po b", p=P)

    with tile.TileContext(nc) as tc:
        with (
            tc.tile_pool(name="wpool", bufs=1) as wpool,
            tc.tile_pool(name="apool", bufs=2) as apool,
            tc.tile_pool(name="gpool", bufs=2) as gpool,
            tc.tile_pool(name="opool", bufs=3) as opool,
            tc.tile_pool(name="pspool", bufs=8, space="PSUM") as pspool,
        ):
            # Weights + bias resident in SBUF for the whole kernel.
            # Host lays out the m columns as [ph=0 gates f,i,ic,o | ph=1 ...]
            # (m = gi + 4*ph), 256 interleaved stationary cols per m.
            wm_sb = wpool.tile([P, KP, MT * 256], E4, tag="wm")
            nc.sync.dma_start(out=wm_sb, in_=wm3)
            wc_sb = wpool.tile([P, KT, MT * 256], E4, tag="wc")
            nc.sync.dma_start(out=wc_sb, in_=wc3)
            bias_sb = wpool.tile([P, MT], F32)
            nc.sync.dma_start(out=bias_sb, in_=biasd)

            # Dummy Silu: forces the ACT table loader to pick the
            # silu_and_others set (the only one containing BOTH Tanh and
            # Sin), so the whole kernel needs exactly one table load.
            if lead_silu:
                dummy = wpool.tile([P, 1], F32)
                nc.vector.memset(dummy, 0.0)
                nc.scalar.activation(dummy, dummy, ACT.Silu)

            def body():
                for nb in range(NBCH):
                    bsl = bass.ds(nb * NB, NB)
                    xc = apool.tile([P, KX, 2, NB], E4, tag="xc")
                    nc.sync.dma_start(out=xc, in_=xq4[:, :, :, bsl])
                    hc = apool.tile([P, KH, 2, NB], E4, tag="hc")
                    nc.sync.dma_start(out=hc, in_=hq4[:, :, :, bsl])
                    cc = apool.tile([P, 2, NB], F32, tag="cc")
                    nc.sync.dma_start(out=cc, in_=cT3[:, :, bsl])

                    for ph in range(2):  # H-slice half (two 128-row m-tiles)
                        ps = []
                        for gi in range(4):  # f, i, ic, o
                            mcol = gi + 4 * ph
                            msl = bass.ds(mcol * 256, 256)
                            pt = pspool.tile([P, NB], F32, tag="ps")
                            # main X@W': swil pair stationaries over k-tiles
                            for kp in range(KP):
                                rhs = (xc[:, 2 * kp:2 * kp + 2, 1, :]
                                       if kp < KX // 2 else
                                       hc[:, 2 * (kp - KX // 2):
                                          2 * (kp - KX // 2) + 2, 1, :])
                                nc.tensor.matmul(
                                    pt, lhsT=wm_sb[:, kp, msl], rhs=rhs,
                                    start=(kp == 0), stop=False,
                                    perf_mode=SWIL,
                                )
                            # correction S@W' + X@R: per k-tile, swil pairs
                            # plane rows (W'.S ; R.X)
                            for kt in range(KT):
                                rhs = (xc[:, kt, :, :] if kt < KX
                                       else hc[:, kt - KX, :, :])
                                nc.tensor.matmul(
                                    pt, lhsT=wc_sb[:, kt, msl], rhs=rhs,
                                    start=False, stop=(kt == KT - 1),
                                    perf_mode=SWIL,
                                )
                            ps.append(pt)

                        cols = [gi + 4 * ph for gi in range(4)]
                        PI, TWO_PI = float(np.pi), float(2 * np.pi)
                        ft = gpool.tile([P, NB], F32, tag="ft")
                        it = gpool.tile([P, NB], F32, tag="it")
                        gt = gpool.tile([P, NB], F32, tag="gt")
                        ot = gpool.tile([P, NB], F32, tag="ot")
                        # sigmoid(z+b) = 0.5*tanh(0.5z + 0.5b) + 0.5 with
                        # z = psum/kappa (bias column pre-scaled by 0.5)
                        nc.scalar.activation(ft, ps[0], ACT.Tanh,
                                             bias=bias_sb[:, cols[0]:cols[0] + 1],
                                             scale=0.5 / KAPPA)
                        nc.scalar.activation(it, ps[1], ACT.Tanh,
                                             bias=bias_sb[:, cols[1]:cols[1] + 1],
                                             scale=0.5 / KAPPA)
                        # ACT Sin is only valid on [-pi, pi]; wrap the
                        # kappa-scaled psum by one kappa*2pi period on the DVE
                        # (sin-gate bias column pre-scaled by kappa), then
                        # descale inside the ACT.
                        gw = gpool.tile([P, NB], F32, tag="gw")
                        nc.vector.add_range_wrap(
                            gw, ps[2], bias_sb[:, cols[2]:cols[2] + 1],
                            KAPPA * PI, KAPPA * TWO_PI)
                        nc.scalar.activation(gt, gw, ACT.Sin, scale=1.0 / KAPPA)
                        nc.scalar.activation(ot, ps[3], ACT.Tanh,
                                             bias=bias_sb[:, cols[3]:cols[3] + 1],
                                             scale=0.5 / KAPPA)
                        MUL, ADD = mybir.AluOpType.mult, mybir.AluOpType.add
                        nc.vector.tensor_scalar(ft, ft, 0.5, 0.5, MUL, ADD)
                        nc.vector.tensor_scalar(it, it, 0.5, 0.5, MUL, ADD)
                        nc.vector.tensor_scalar(ot, ot, 0.5, 0.5, MUL, ADD)

                        ctn = opool.tile([P, NB], F32, tag="ctn")
                        tmp = opool.tile([P, NB], F32, tag="tmp")
                        nc.vector.tensor_mul(ctn, cc[:, ph, :], ft)
                        nc.vector.tensor_mul(tmp, gt, it)
                        nc.vector.tensor_add(ctn, ctn, tmp)
                        cw = opool.tile([P, NB], F32, tag="cw")
                        nc.vector.add_range_wrap(cw, ctn, 0.0, PI, TWO_PI)
                        sct = opool.tile([P, NB], F32, tag="sct")
                        nc.scalar.activation(sct, cw, ACT.Sin)
                        htn = opool.tile([P, NB], F32, tag="htn")
                        nc.vector.tensor_mul(htn, ot, sct)
                        nc.sync.dma_start(out=ctT3[:, ph, bsl], in_=ctn)
                        nc.sync.dma_start(out=htT3[:, ph, bsl], in_=htn)

            if repeats == 1:
                body()
            else:
                with tc.For_i(0, repeats, 1):
                    body()

            if internal_io:
                done = nc.dram_tensor("done", [P, MT], F32,
                                      kind="ExternalOutput").ap()
                dtile = wpool.tile([P, MT], F32)
                nc.vector.tensor_copy(dtile, bias_sb)
                nc.sync.dma_start(out=done, in_=dtile)

    nc.compile()
    return nc


def _get_module(repeats: int = 1) -> "bacc.Bacc":
    if repeats not in _MODULES:
        _MODULES[repeats] = _build_module(repeats)
    return _MODULES[repeats]


def _quant_pair(a: np.ndarray, scale: float):
    """Return (residual, main) e4m3 planes of scale*a: X = e4(scale*a),
    S = e4(scale*a - X)."""
    sa = np.asarray(a, np.float32) * np.float32(scale)
    X = sa.astype(E4NP)
    S = (sa - X.astype(np.float32)).astype(E4NP)
    return S, X


def _interleave(p0: np.ndarray, p1: np.ndarray, kt: int):
    """[kt*128, N] planes -> [kt*2*128, N] rows ordered (k, plane, p).
    Acts: p0 = S residual, p1 = X main. Weights: p0 = W' main, p1 = R."""
    n = p0.shape[1]
    out = np.empty((kt, 2, P, n), E4NP)
    out[:, 0] = p0.reshape(kt, P, n)
    out[:, 1] = p1.reshape(kt, P, n)
    return np.ascontiguousarray(out.reshape(kt * 2 * P, n))


def make_in_maps(x, h, c, w_ih, w_hh, b_ih, b_hh):
    """Host-side shard + transpose + fp8 split. Returns per-core input maps."""
    x = np.asarray(x, np.float32)
    h = np.asarray(h, np.float32)
    c = np.asarray(c, np.float32)
    w_ih = np.asarray(w_ih, np.float32)
    w_hh = np.asarray(w_hh, np.float32)
    bias = np.asarray(b_ih, np.float32) + np.asarray(b_hh, np.float32)

    # Activations: shared by all cores. Planes 0/1 = S/X at scale AQ.
    xS, xX = _quant_pair(x.T, AQ)            # [IN, B]
    hS, hX = _quant_pair(h.T, AQ)            # [H, B]
    xq = _interleave(xS, xX, KX)
    hq = _interleave(hS, hX, KH)
    cTt = np.ascontiguousarray(c.T)          # [H, B]

    # Weights: quantize the full transposed matrices once, slice per core.
    wihR, wihW = _quant_pair(w_ih.T, WQ)     # [IN, 4H] main in wihW
    whhR, whhW = _quant_pair(w_hh.T, WQ)     # [H, 4H]

    # m-tile bias scale: 0.5 for tanh-based sigmoid gates (f,i,o), KAPPA for
    # the sin gate (ic) whose wrap runs on kappa-scaled psum values.
    # m ordering is [f0,i0,ic0,o0, f1,i1,ic1,o1] (m = gate + 4*half)
    mscale = np.array([0.5, 0.5, KAPPA, 0.5] * 2, np.float32)

    in_maps = []
    for core in range(NCORES):
        cols = np.concatenate(
            [gate * H + core * SH + half * P + np.arange(P)
             for half in range(2) for gate in range(4)])
        # Full per-core [K=3072, G] main/residual planes, k-tiles x then h.
        Wall = np.concatenate([wihW[:, cols], whhW[:, cols]], axis=0)
        Rall = np.concatenate([wihR[:, cols], whhR[:, cols]], axis=0)
        W4 = Wall.reshape(KT, P, MT, P)              # [kt, p, m, j]
        R4 = Rall.reshape(KT, P, MT, P)
        # SwInterleave stored layout per 256-col block: A/B column-pairs
        # interleaved, columns reversed: st[:,2u]=A[:,127-u], st[:,2u+1]=B.
        wmh = np.empty((KP, P, MT, 2 * P), E4NP)
        wmh[..., 0::2] = W4[0::2][..., ::-1]         # A = W'[2kp]
        wmh[..., 1::2] = W4[1::2][..., ::-1]         # B = W'[2kp+1]
        wch = np.empty((KT, P, MT, 2 * P), E4NP)
        wch[..., 0::2] = W4[..., ::-1]               # A = W'[kt] (pairs S)
        wch[..., 1::2] = R4[..., ::-1]               # B = R[kt]  (pairs X)
        b_c = bias[cols]                             # [G]
        bias_mat = np.ascontiguousarray(
            (b_c.reshape(MT, P) * mscale[:, None]).T)  # [P, MT]
        in_maps.append({
            "xq": xq,
            "hq": hq,
            "cT": np.ascontiguousarray(cTt[core * SH:(core + 1) * SH]),
            "wm": np.ascontiguousarray(wmh.reshape(KP * P, MT * 256)),
            "wc": np.ascontiguousarray(wch.reshape(KT * P, MT * 256)),
            "biasd": bias_mat,
        })
    return in_maps


def assemble_outputs(results):
    """results: per-core dicts with htT/ctT [SH, B] -> full (ht, ct)."""
    htT = np.concatenate([results[c]["htT"] for c in range(NCORES)], axis=0)
    ctT = np.concatenate([results[c]["ctT"] for c in range(NCORES)], axis=0)
    ht = np.ascontiguousarray(htT.T)
    ct = np.ascontiguousarray(ctT.T)
    return ht, ct


def kernel(x, h, c, w_ih, w_hh, b_ih, b_hh):
    nc = _get_module(repeats=1)
    in_maps = make_in_maps(x, h, c, w_ih, w_hh, b_ih, b_hh)
    res = run_bass_kernel_spmd(nc, in_maps, core_ids=list(range(NCORES)))
    return assemble_outputs(res.results)
